# revision 92
# baseline (speedup 1.0000x reference)
"""Trainium2 Bass kernel for nn_BetweenClusterFC.

Computes out[e] = (emb_1[f[e]] @ W1 + b1) . (emb_2[t[e]] @ W2 + b2)
for E = 1.6M edges over N = 100k nodes, D_IN = 256, D_OUT = 128.

Strategy (8 NeuronCores, SPMD, full inputs in / full output out):
  - Nodes are split into 8 blocks of 12500.  Edges are assigned to cores by a
    (from-block-group, to-block-group) 4x2 rectangle: core c=(a,b) handles
    edges with from-node in blocks [4a..4a+3] and to-node in blocks
    [2b..2b+1].  Uniform (~200k edges/core); each core needs projections for
    4 from-blocks + 2 to-blocks (75k nodes).
  - All streamed data is bf16: the host pre-transposes and casts the
    embedding shards; the PE projects p = emb @ W (+ bias folded in as a
    K=1 ones x bias matmul), the Act engine moves psum -> sbuf in bf16 and
    issues the p-table writes (1KB-contiguous via a (p t)-permuted row
    order), keeping the DVE entirely free for the edge phase.
  - Edges are bucketed host-side by (local from-block, local to-block); per
    bucket both endpoint rows are fetched with SWDGE dma_gather calls (1024
    idxs/call HW limit, two DMA queues, rows moved as packed f32 pairs)
    using int16 permuted-local indices.  Calls land in contiguous 4096-edge
    "items" (two full gather pairs sharing one of 5 rotating super-slots);
    the DVE runs a software-pipelined bf16 multiply (2x mode) + bf16
    tree-halving + short f32 reduce per item into a single SBUF-resident
    result tile, written out in two halves.
  - Bucket capacities are sized from the actual per-core edge counts (max
    across cores, 128-aligned), so the Bass program is built per problem
    instance (inspector-executor style) and cached.  Bucket 0 is ordered
    early-prefix-rows first so its first gather pairs unblock after 38 of
    150 projection groups; the idle pre-gather Pool engine prefetches the
    idx tables and part of the embT stream; the DVE covers half the early
    psum->sbuf copies; p-table writes go out as two-group pair DMAs to
    amortize the per-DMA descriptor-generation floor.
  - The host applies the inverse edge permutation to assemble the output.

Written in raw Bass (explicit semaphores) - the Tile layer's generated sync
exceeds this toolchain's per-instruction wait-slot limits.
"""

import contextlib

import numpy as np

import concourse.bass as bass
import concourse.mybir as mybir

# ---------------------------------------------------------------- constants
N_NODES = 100_000
D_IN = 256
D_OUT = 128
N_EDGES = 1_600_000
N_CORES = 8

NB = 12_500          # nodes per block
NBP = 12_800         # padded block rows (25 * 512) - multiple of 512
NFB = 4              # from-blocks per core
NTB = 2              # to-blocks per core
NBUCKET = NFB * NTB  # 8 buckets per core

P1_ROWS = NFB * NBP  # 51200
P2_ROWS = NTB * NBP  # 25600

TILES1 = P1_ROWS // 128    # 400 node-tiles, table 1
TILES2 = P2_ROWS // 128    # 200 node-tiles, table 2
GROUPS1 = TILES1 // 4      # 100 psum groups
GROUPS2 = TILES2 // 4      # 50
NGROUP = GROUPS1 + GROUPS2  # 150
CHUNK_T = 20               # node-tiles per embT load chunk (= 5 groups)
NCH1 = TILES1 // CHUNK_T   # 20 chunks
NCH2 = TILES2 // CHUNK_T   # 10
NCHUNK = NCH1 + NCH2       # 30
EMB_COLS = CHUNK_T * 128   # 2560

MAX_CALL = 1024            # dma_gather idxs per call (HW limit; ring 4096/queue)
HROWS = 6_656              # p1-block-0 prefix rows (13 groups; pair-even gate)
PH1 = 52                   # DVE assists psum->pv copies for odd groups < PH1

F32 = mybir.dt.float32
BF16 = mybir.dt.bfloat16
I16 = mybir.dt.int16
AX = mybir.AxisListType


# Projection block order: interleave p2/p1 blocks so gather buckets unblock
# as early as possible (bucket (fi,ti) needs p1 block fi + p2 block ti):
# (0,0)@50 groups, (0,1)@75, (1,*)@100, (2,*)@125, (3,*)@150.
BSEQ = [(1, 0), (0, 0), (1, 1), (0, 1), (0, 2), (0, 3)]
GPB = NBP // 512   # 25 groups per block
CPB = GPB // 5     # 5 chunks per block (CHUNK_T = 4 groups... 20 tiles)
GSEQ = []
CSEQ = []
for tab, blk in BSEQ:
    g0 = blk * GPB + (GROUPS1 if tab == 1 else 0)
    GSEQ += list(range(g0, g0 + GPB))
    c0 = blk * 5 + (NCH1 if tab == 1 else 0)
    CSEQ += list(range(c0, c0 + 5))
CPOS = {cid: q for q, cid in enumerate(CSEQ)}
_BPOS = {tb: i for i, tb in enumerate(BSEQ)}
# bucket (fi,ti) ready after this many groups in GSEQ order
BK_READY_Q = [GPB * (1 + max(_BPOS[(0, fi)], _BPOS[(1, ti)]))
              for fi in range(NFB) for ti in range(NTB)]


def _chunk_of_tile(tg):
    """global tile index -> (global chunk id, local col0 within chunk)."""
    if tg < TILES1:
        c = tg // CHUNK_T
        return c, (tg % CHUNK_T) * 128
    t2 = tg - TILES1
    c = NCH1 + t2 // CHUNK_T
    return c, (t2 % CHUNK_T) * 128


def _chunk_src(c):
    """global chunk id -> (table, col0)."""
    if c < NCH1:
        return 0, c * EMB_COLS
    return 1, (c - NCH1) * EMB_COLS


def _plan_calls(caps):
    """caps: per-bucket slot capacities (multiples of 128).
    Returns a list of gather-call PAIRS (bucket, n0, n1, idx_col0, slot_col0):
    two <=1024-idx gather calls whose outputs land adjacently so the DVE can
    process 2048 edges per op chain (n1 == 0 for a lone remainder call).
    Plus (idx_cols_total, slot_cols_total, per-bucket slot offsets)."""
    pairs = []
    icol = 0
    scol = 0
    boff = []
    for bk, cap in enumerate(caps):
        boff.append(scol)
        left = cap
        while left > 0:
            n0 = min(MAX_CALL, left)
            n1 = min(MAX_CALL, left - n0)
            pairs.append((bk, n0, n1, icol, scol))
            icol += (n0 + n1) // 16
            scol += (n0 + n1) // 128
            left -= n0 + n1
    return pairs, icol, scol, boff


# ---------------------------------------------------------------- device code
def build_bass(caps, ep0):
    """caps: per-bucket capacities (edge slots, multiples of 128), shared by
    all cores (max over cores).  ep0: bucket-0 pairs gathering only the
    HROWS-row prefix of p1 block 0 (host orders those edges first)."""
    calls, idx_cols, tot_slots, boff = _plan_calls(caps)
    ncall = len(calls)

    nc = bass.Bass(dynamic_dma_scratch_size=32768, num_swdge_queues=2)

    e1t = nc.dram_tensor("e1t", [D_IN, P1_ROWS], BF16, kind="ExternalInput")
    e2t = nc.dram_tensor("e2t", [D_IN, P2_ROWS], BF16, kind="ExternalInput")
    w12 = nc.dram_tensor("w12", [D_IN, 2 * D_OUT], BF16, kind="ExternalInput")
    bo3 = nc.dram_tensor("bo3", [3, D_OUT], BF16, kind="ExternalInput")
    idxa = nc.dram_tensor("idxa", [128, idx_cols], I16, kind="ExternalInput")
    idxb = nc.dram_tensor("idxb", [128, idx_cols], I16, kind="ExternalInput")
    res = nc.dram_tensor("res", [128, tot_slots], F32, kind="ExternalOutput")

    p1d = nc.dram_tensor("p1d", [P1_ROWS, D_OUT], BF16, kind="Internal")
    p2d = nc.dram_tensor("p2d", [P2_ROWS, D_OUT], BF16, kind="Internal")
    pdst = (p1d, p2d)

    st = contextlib.ExitStack()
    with st:
        sb = lambda nm, shape, dt: st.enter_context(nc.sbuf_tensor(nm, shape, dt))
        sem = lambda nm: st.enter_context(nc.semaphore(name=nm))

        # wc columns: [w1_k0 | w2_k0 | w1_k1 | w2_k1] (128 each)
        wc = sb("wc", [128, 512], BF16)
        bt = [sb(f"bt{i}", [1, 128], BF16) for i in range(3)]  # b1, b2, ones
        idxt = (sb("idxta", [128, idx_cols], I16), sb("idxtb", [128, idx_cols], I16))
        et = [[sb(f"et_{p}_{k}", [128, EMB_COLS], BF16) for k in range(2)]
              for p in range(3)]  # [buffer cq%3][k-half]
        pvt = sb("pvt", [128, 4 * 512], BF16)
        pv = [pvt[:, i * 512:(i + 1) * 512] for i in range(4)]
        ps = [st.enter_context(nc.psum_tensor(f"ps{i}", [128, 512], F32))
              for i in range(4)]
        # Gather pairs are merged into "items" of up to two full pairs (4096
        # edges) sharing one contiguous super-slot, so each DVE chain op's
        # fixed cost is amortized over twice the edges.  4 rotating slots.
        NBUF = 5
        ITEM_COLS = 4 * MAX_CALL
        atb = sb("atb", [128, NBUF * ITEM_COLS], BF16)
        btgb = sb("btgb", [128, NBUF * ITEM_COLS], BF16)
        rt = sb("rt", [128, tot_slots], F32)

        s_cl = sem("s_cl")               # w const loads (2 dmas -> 32)
        s_clb = sem("s_clb")             # bias/ones const loads (3 -> 48)
        s_cli = sem("s_cli")             # idx table loads (2 dmas -> 32)
        s_load = tuple(sem(f"s_load{i}") for i in range(3))  # SP embT, by cq%3
        s_loadp = sem("s_loadp")         # Pool embT loads
        s_mm = sem("s_mm")               # matmuls (+1 each; 3 per tile)
        s_cp = sem("s_cp")               # act copies (+1 each)
        s_cpd = sem("s_cpd")             # dve copies (+1 each)
        s_pw = tuple(sem(f"s_pw{i}") for i in range(2))  # pair writes, by j%2
        s_pwx = sem("s_pwx")             # first halves of crossing pairs
        # gather-completion sems by item%NBUF per queue: item w+NBUF's gathers
        # are gated on s_red >= w+1, so updates of each sem are causally
        # ordered.
        s_ga = tuple(sem(f"s_ga{i}") for i in range(NBUF))  # queue-0, +16/call
        s_gb = tuple(sem(f"s_gb{i}") for i in range(NBUF))  # queue-1, +16/call
        s_st = tuple(sem(f"s_st{i}") for i in range(4))  # DVE dot stages 0-3
        s_red = sem("s_red")             # final reduces (+1 per item)
        s_out = sem("s_out")             # res dma

        CONSTS = 2 * 16

        # items: (pair list, slot columns per pair member, total slots, scol)
        items = []
        _p = 0
        while _p < ncall:
            bk0, n0, n1, icol0, scol0 = calls[_p]
            if (n0 + n1 == 2 * MAX_CALL and _p + 1 < ncall
                    and calls[_p + 1][0] == bk0):
                items.append(([_p, _p + 1], scol0))
                _p += 2
            else:
                items.append(([_p], scol0))
                _p += 1
        nitem = len(items)
        item_slots = [sum((calls[p][1] + calls[p][2]) // 128 for p in ps_)
                      for ps_, _ in items]

        # per-sem cumulative gather counts per item
        _sem_cnt = [0] * NBUF
        gwait = []
        for _w, (_ps, _sc) in enumerate(items):
            _sem_cnt[_w % NBUF] += 16 * sum(
                2 if calls[p][2] else 1 for p in _ps)
            gwait.append(_sem_cnt[_w % NBUF])

        # During the pre-gather phase the DVE is mostly idle: it takes the odd
        # groups' psum->pv copies for the first PH1 groups.
        dve_copy = [q < PH1 and q % 2 == 1 for q in range(NGROUP)]
        cp_cnt = []  # cumulative (act, dve) copy counts through group q
        _na = _ndv = 0
        for q in range(NGROUP):
            if dve_copy[q]:
                _ndv += 1
            else:
                _na += 1
            cp_cnt.append((_na, _ndv))

        def _copy_wait(eng, q):
            """wait until the copy of group q is complete."""
            if dve_copy[q]:
                eng.wait_ge(s_cpd, cp_cnt[q][1])
            else:
                eng.wait_ge(s_cp, cp_cnt[q][0])

        block = st.enter_context(nc.Block())

        # Chunks 1,3,5,7,9 are loaded by the Pool engine during its pre-gather
        # idle window (own sem s_loadp); SP loads the rest (s_load by parity).
        POOL_CHUNKS = (4, 6, 8)
        chunk_wait = {}
        _cnt = {0: 0, 1: 0, 2: 0, 'p': 0}
        for _cq in range(NCHUNK):
            key = 'p' if _cq in POOL_CHUNKS else _cq % 3
            _cnt[key] += 32
            chunk_wait[_cq] = (key, _cnt[key])

        # p-writes happen per PAIR of groups (2j, 2j+1): pv slots are adjacent
        # so one DMA covers both, halving Act's per-write descriptor-gen
        # floor.  Pairs whose two groups land in different projection blocks
        # ("crossing": non-contiguous table rows) are written as two singles,
        # the first tracked by s_pwx.
        NPAIR_W = NGROUP // 2
        CROSSING = frozenset(j for j in range(NPAIR_W)
                             if GSEQ[2 * j + 1] != GSEQ[2 * j] + 1)
        _xrank = {j: i + 1 for i, j in enumerate(sorted(CROSSING))}

        def _one_write(eng, q, s_sem, cnt):
            g = GSEQ[q]
            tab = 0 if g < GROUPS1 else 1
            r0 = g * 512 if tab == 0 else (g - GROUPS1) * 512
            # (p t)-permuted write: table row r0 + p*4 + t <- node r0 + t*128 + p
            eng.dma_start(
                out=pdst[tab][r0:r0 + 512, :].rearrange("(p t) d -> p t d", p=128),
                in_=pv[q % 4][:].rearrange("p (t d) -> p t d", d=128),
            ).then_inc(s_sem, cnt)

        def p_write_pair(eng, j):
            if j in CROSSING:
                _one_write(eng, 2 * j, s_pwx, 16)
                _one_write(eng, 2 * j + 1, s_pw[j % 2], 16)
                return
            q0 = 2 * j
            g = GSEQ[q0]
            tab = 0 if g < GROUPS1 else 1
            r0 = g * 512 if tab == 0 else (g - GROUPS1) * 512
            s0 = q0 % 4
            eng.dma_start(
                out=pdst[tab][r0:r0 + 1024, :]
                    .rearrange("(g p t) d -> p g t d", p=128, t=4),
                in_=pvt[:, s0 * 512:(s0 + 2) * 512]
                    .rearrange("p (g t d) -> p g t d", t=4, d=128),
            ).then_inc(s_pw[j % 2], 16)

        def wait_pairs_through(eng, nq):
            """wait until all p-writes for groups < nq are complete (nq is
            rounded up to a pair boundary)."""
            npair = (nq + 1) // 2
            for r in range(2):
                eng.wait_ge(s_pw[r], 16 * len(range(r, npair, 2)))
            nx = sum(1 for j in CROSSING if j < npair)
            if nx:
                eng.wait_ge(s_pwx, 16 * nx)

        def wait_pv_drained(eng, q):
            """wait until pv slot q%4 (last used by group q-4) is rewritable."""
            p4 = (q - 4) // 2
            eng.wait_ge(s_pw[p4 % 2], 16 * (p4 // 2 + 1))
            if p4 in CROSSING:
                eng.wait_ge(s_pwx, 16 * _xrank[p4])

        def load_chunk(eng, cq):
            if cq >= 3:
                # buffer cq%3 previously held chunk cq-3; wait consumed
                eng.wait_ge(s_mm, 3 * CHUNK_T * (cq - 2))
            tab, col0 = _chunk_src(CSEQ[cq])
            src = e1t if tab == 0 else e2t
            par = cq % 3
            s_ld = s_loadp if cq in POOL_CHUNKS else s_load[cq % 3]
            eng.dma_start(out=et[par][0][:],
                          in_=src[0:128, col0:col0 + EMB_COLS]).then_inc(s_ld, 16)
            eng.dma_start(out=et[par][1][:],
                          in_=src[128:256, col0:col0 + EMB_COLS]).then_inc(s_ld, 16)

        # ------------------------------------------------ SP: embT + p-writes
        @block.sync
        def _(sync):
            for k in range(2):
                sync.dma_start(out=wc[:, k * 256:(k + 1) * 256],
                               in_=w12[k * 128:(k + 1) * 128, :]).then_inc(s_cl, 16)
            load_chunk(sync, 0)
            for i in range(3):
                sync.dma_start(out=bt[i][:], in_=bo3[i:i + 1, :]).then_inc(s_clb, 16)
            load_chunk(sync, 1)
            next_cq = 2
            for q in range(NGROUP):
                # look ahead: issue loads for chunks starting within 5 groups
                while next_cq < NCHUNK and next_cq * CHUNK_T <= (q + 5) * 4 + 3:
                    if next_cq not in POOL_CHUNKS:
                        load_chunk(sync, next_cq)
                    next_cq += 1


        # ------------------------------------------------ PE: projections
        @block.tensor
        def _(tensor):
            tensor.wait_ge(s_cl, CONSTS)
            for q, g in enumerate(GSEQ):
                tab = 0 if g < GROUPS1 else 1
                bc = bt[tab][:]
                for j in range(4):
                    tq = q * 4 + j
                    cid, col0 = _chunk_of_tile(g * 4 + j)
                    cq = CPOS[cid]
                    if tq == cq * CHUNK_T:  # first processed tile of chunk
                        key, cntv = chunk_wait[cq]
                        tensor.wait_ge(s_loadp if key == 'p' else s_load[key], cntv)
                    if j == 0 and q >= 4:
                        _copy_wait(tensor, q - 4)  # psum bank q%4 free
                    out = ps[q % 4][:, j * 128:(j + 1) * 128]
                    tensor.matmul(out=out, lhsT=et[cq % 3][0][:, col0:col0 + 128],
                                  rhs=wc[:, tab * 128:tab * 128 + 128],
                                  start=True, stop=False).then_inc(s_mm, 1)
                    tensor.matmul(out=out, lhsT=et[cq % 3][1][:, col0:col0 + 128],
                                  rhs=wc[:, 256 + tab * 128:256 + tab * 128 + 128],
                                  start=False, stop=False).then_inc(s_mm, 1)
                    if tq == 0:
                        tensor.wait_ge(s_clb, 48)
                    tensor.matmul(out=out, lhsT=bt[2][:], rhs=bc,
                                  start=False, stop=True).then_inc(s_mm, 1)

        # ------------------------------------------------ Act: psum->sbuf + DMAs
        @block.scalar
        def _(scalar):
            scalar.wait_ge(s_cl, CONSTS)
            # the pair write of groups (q-2, q-1) is issued after the copy of
            # group q so the copy-completion sems are already visible.
            for q, g in enumerate(GSEQ):
                if not dve_copy[q]:
                    scalar.wait_ge(s_mm, 12 * q + 12)
                    if q >= 4:
                        wait_pv_drained(scalar, q)
                    scalar.copy(out=pv[q % 4][:], in_=ps[q % 4][:]).then_inc(s_cp, 1)
                if q >= 2 and q % 2 == 0:
                    _copy_wait(scalar, q - 2)
                    _copy_wait(scalar, q - 1)
                    p_write_pair(scalar, (q - 2) // 2)
            _copy_wait(scalar, NGROUP - 2)
            _copy_wait(scalar, NGROUP - 1)
            p_write_pair(scalar, NPAIR_W - 1)
            # write results in two halves so only the last sliver is exposed
            half_w = nitem // 2
            half_s = items[half_w][1]
            scalar.wait_ge(s_red, half_w)
            scalar.dma_start(out=res[:, :half_s], in_=rt[:, :half_s]).then_inc(s_out, 16)
            scalar.wait_ge(s_red, nitem)
            scalar.dma_start(out=res[:, half_s:], in_=rt[:, half_s:]).then_inc(s_out, 16)
            scalar.wait_ge(s_out, 32)

        # ------------------------------------------------ Pool: gathers
        @block.gpsimd
        def _(gpsimd):
            # the Pool engine is idle until the first bucket gate (~40us): it
            # loads the consts, every other early embT chunk, and its own idx
            # tables, so SP's chunk stream (which feeds the PE) never blocks.
            gpsimd.dma_start(out=idxt[0][:], in_=idxa[:]).then_inc(s_cli, 16)
            gpsimd.dma_start(out=idxt[1][:], in_=idxb[:]).then_inc(s_cli, 16)
            for i, cq in enumerate(POOL_CHUNKS):
                if i:
                    gpsimd.wait_ge(s_loadp, 32 * i)  # order s_loadp updates
                load_chunk(gpsimd, cq)
            from concourse import library_config
            gpsimd.load_library(library_config.mlp)
            sizes = ({c[1] for c in calls} | {c[2] for c in calls}) - {0}
            regs = {n: gpsimd.to_reg(n) for n in sorted(sizes)}
            gpsimd.wait_ge(s_cl, CONSTS)
            gpsimd.wait_ge(s_cli, 32)

            # first ep0 pairs of bucket 0 reference only p1 rows < HROWS (the
            # host orders bucket-0 edges early-rows-first), so they unblock
            # after 25 + HROWS/512 projected groups instead of 50.
            gate_req = []
            for p, (bk, n0, n1, icol, scol) in enumerate(calls):
                if bk == 0 and p < ep0:
                    gate_req.append(GPB + HROWS // 512)
                else:
                    gate_req.append(BK_READY_Q[bk])

            cur_gate = -1
            for w, (ps_, scol_w) in enumerate(items):
                if w >= NBUF:
                    gpsimd.wait_ge(s_red, w - NBUF + 1)  # slot w%NBUF consumed
                base = (w % NBUF) * ITEM_COLS
                nh = 0
                for p in ps_:
                    bk, n0, n1, icol, scol = calls[p]
                    fi, ti = bk // NTB, bk % NTB
                    if gate_req[p] > cur_gate:
                        cur_gate = gate_req[p]
                        wait_pairs_through(gpsimd, cur_gate)
                    rows1 = HROWS if (bk == 0 and p < ep0) else NBP
                    # rows move as 64 packed f32 (= 128 bf16): same bytes and
                    # descriptors, half the modeled element count.
                    for n_i, n in enumerate((n0, n1)):
                        if n == 0:
                            continue
                        S = n // 128
                        c0 = icol + (nh % (2 * MAX_CALL)) // 16
                        gpsimd.dma_gather(
                            out_ap=atb[:, base + nh:base + nh + S * 128]
                                .bitcast(F32).rearrange("p (s d) -> p s d", d=64),
                            in_ap=p1d[fi * NBP:fi * NBP + rows1, :].bitcast(F32),
                            idxs_ap=idxt[0][:, c0:c0 + n // 16],
                            num_idxs=n, num_idxs_reg=regs[n],
                            elem_size=D_OUT // 2, queue_num=0,
                        ).then_inc(s_ga[w % NBUF], 16)
                        gpsimd.dma_gather(
                            out_ap=btgb[:, base + nh:base + nh + S * 128]
                                .bitcast(F32).rearrange("p (s d) -> p s d", d=64),
                            in_ap=p2d[ti * NBP:(ti + 1) * NBP, :].bitcast(F32),
                            idxs_ap=idxt[1][:, c0:c0 + n // 16],
                            num_idxs=n, num_idxs_reg=regs[n],
                            elem_size=D_OUT // 2, queue_num=1,
                        ).then_inc(s_gb[w % NBUF], 16)
                        nh += S * 128

        # ------------------------------------------------ DVE: dot products
        # bf16 multiply (2x mode), tree-halve in bf16 (2x) down to 16 partials
        # per slot, then one short f32 reduce - one chain per gather pair
        # (up to 2048 edges).  The 5-stage chain is software-pipelined across
        # pairs (stage s of pair p in round p+s) so every intra-chain
        # semaphore is already visible when its wait issues.
        @block.vector
        def _(vector):
            # phase-1 assist: odd-group psum->pv copies while gathers are gated
            for q in range(NGROUP):
                if not dve_copy[q]:
                    continue
                vector.wait_ge(s_mm, 12 * q + 12)
                if q >= 4:
                    wait_pv_drained(vector, q)
                vector.tensor_copy(out=pv[q % 4][:], in_=ps[q % 4][:]).then_inc(s_cpd, 1)

            def stage(s, w):
                S = item_slots[w]
                scol_w = items[w][1]
                base = (w % NBUF) * ITEM_COLS
                a3 = atb[:, base:base + S * 128]
                av = a3.rearrange("p (s d) -> p s d", d=128)
                if s == 0:
                    vector.wait_ge(s_ga[w % NBUF], gwait[w])
                    vector.wait_ge(s_gb[w % NBUF], gwait[w])
                    vector.tensor_mul(out=a3, in0=a3,
                                      in1=btgb[:, base:base + S * 128]
                                      ).then_inc(s_st[0], 1)
                elif s in (1, 2, 3):
                    h = 128 >> s  # 64, 32, 16
                    vector.wait_ge(s_st[s - 1], w + 1)
                    vector.tensor_add(out=av[:, :, 0:h], in0=av[:, :, 0:h],
                                      in1=av[:, :, h:2 * h]).then_inc(s_st[s], 1)
                else:
                    vector.wait_ge(s_st[3], w + 1)
                    vector.reduce_sum(out=rt[:, scol_w:scol_w + S],
                                      in_=av[:, :, 0:16], axis=AX.X).then_inc(s_red, 1)

            for r in range(nitem + 4):
                for s in range(4, -1, -1):
                    w = r - s
                    if 0 <= w < nitem:
                        stage(s, w)

    return nc, calls, boff


_NC_CACHE: dict = {}


def _get_nc(caps):
    caps, ep0 = caps
    key = (tuple(caps), ep0)
    if key not in _NC_CACHE:
        nc, calls, boff = build_bass(caps, ep0)
        from concourse.library_overlay import lower_extended_insts
        lower_extended_insts(nc)
        _NC_CACHE[key] = (nc, calls, boff)
    return _NC_CACHE[key]


# ---------------------------------------------------------------- host side
def _perm_local(n):
    """block-local node id -> permuted table row (within block).
    Table row g*512 + p*4 + t holds node g*512 + t*128 + p."""
    g, u = np.divmod(n, 512)
    t, p = np.divmod(u, 128)
    return g * 512 + p * 4 + t


def _marshal(emb_1, emb_2, nodes_from_to, W1, b1, W2, b2):
    """Shard/bucket inputs per core.  Returns (caps, in_maps, books)."""
    import ml_dtypes
    bf16 = ml_dtypes.bfloat16

    f = np.asarray(nodes_from_to[:, 0], dtype=np.int64)
    t = np.asarray(nodes_from_to[:, 1], dtype=np.int64)
    emb_1 = np.asarray(emb_1, dtype=np.float32)
    emb_2 = np.asarray(emb_2, dtype=np.float32)
    w12 = np.concatenate(
        [np.asarray(W1, dtype=np.float32), np.asarray(W2, dtype=np.float32)],
        axis=1).astype(bf16)
    bo3 = np.stack([
        np.asarray(b1, dtype=np.float32).reshape(-1),
        np.asarray(b2, dtype=np.float32).reshape(-1),
        np.ones(D_OUT, np.float32),
    ]).astype(bf16)

    core = (f // (NFB * NB)) * 4 + t // (NTB * NB)
    order0 = np.argsort(core, kind="stable")
    ccnt = np.bincount(core, minlength=N_CORES)
    coff = np.concatenate([[0], np.cumsum(ccnt)])

    percore = []
    early_cnts = []
    all_cnts = np.zeros((N_CORES, NBUCKET), np.int64)
    for c in range(N_CORES):
        a, b = c // 4, c % 4
        sel = order0[coff[c]:coff[c + 1]]
        fc, tcv = f[sel], t[sel]
        fi = fc // NB - NFB * a
        ti = tcv // NB - NTB * b
        fl = _perm_local(fc % NB).astype(np.int16)
        tl = _perm_local(tcv % NB).astype(np.int16)
        bk = fi * NTB + ti
        o2 = np.argsort(bk, kind="stable")
        sel2, fl2, tl2 = sel[o2], fl[o2], tl[o2]
        cnts = np.bincount(bk, minlength=NBUCKET)
        all_cnts[c] = cnts
        # bucket 0: early (prefix-row) edges first, enabling the half-gate
        n0 = cnts[0]
        e0 = fl2[:n0] < HROWS
        o3 = np.argsort(~e0, kind="stable")
        sel2[:n0], fl2[:n0], tl2[:n0] = sel2[:n0][o3], fl2[:n0][o3], tl2[:n0][o3]
        early_cnts.append(int(e0.sum()))
        percore.append((a, b, sel2, fl2, tl2, cnts))

    caps = [int(-(-all_cnts[:, k].max() // 128) * 128) for k in range(NBUCKET)]
    ep0 = min(early_cnts) // (2 * MAX_CALL)
    calls, idx_cols, tot_slots, boff = _plan_calls(caps)

    in_maps, books = [], []
    for c in range(N_CORES):
        a, b, sel2, fl2, tl2, cnts = percore[c]
        pos = np.concatenate([[0], np.cumsum(cnts)])

        slots_a = np.zeros((NBUCKET, max(caps)), np.int16)
        slots_b = np.zeros((NBUCKET, max(caps)), np.int16)
        for k in range(NBUCKET):
            slots_a[k, :cnts[k]] = fl2[pos[k]:pos[k + 1]]
            slots_b[k, :cnts[k]] = tl2[pos[k]:pos[k + 1]]
        # wrap by 16: idx i of a bucket at (partition i%16, col i//16),
        # replicated across the 8 groups of 16 partitions
        wa_cols = []
        wb_cols = []
        for k in range(NBUCKET):
            cap = caps[k]
            wa_cols.append(slots_a[k, :cap].reshape(cap // 16, 16).T)
            wb_cols.append(slots_b[k, :cap].reshape(cap // 16, 16).T)
        idxa = np.tile(np.concatenate(wa_cols, axis=1), (8, 1))
        idxb = np.tile(np.concatenate(wb_cols, axis=1), (8, 1))

        e1t = np.zeros((D_IN, P1_ROWS), bf16)
        for i in range(NFB):
            blk = emb_1[(NFB * a + i) * NB:(NFB * a + i + 1) * NB]
            e1t[:, i * NBP:i * NBP + NB] = blk.T.astype(bf16)
        e2t = np.zeros((D_IN, P2_ROWS), bf16)
        for i in range(NTB):
            blk = emb_2[(NTB * b + i) * NB:(NTB * b + i + 1) * NB]
            e2t[:, i * NBP:i * NBP + NB] = blk.T.astype(bf16)

        in_maps.append({
            "e1t": e1t, "e2t": e2t, "w12": w12, "bo3": bo3,
            "idxa": np.ascontiguousarray(idxa),
            "idxb": np.ascontiguousarray(idxb),
        })
        books.append((sel2, cnts, pos))
    return (caps, ep0), in_maps, books


def _unmarshal(results, books, caps, n_edges):
    calls, idx_cols, tot_slots, boff = _plan_calls(caps[0])
    out = np.empty(n_edges, np.float32)
    for c in range(N_CORES):
        sel2, cnts, pos = books[c]
        r = results[c]["res"]  # [128, tot_slots]
        for k in range(NBUCKET):
            if cnts[k] == 0:
                continue
            s0 = boff[k]
            nslots = caps[0][k] // 128
            stream = r[:, s0:s0 + nslots].T.reshape(-1)
            out[sel2[pos[k]:pos[k + 1]]] = stream[:cnts[k]]
    return out


def _run(inputs, trace=False, **run_kwargs):
    from concourse.bass_utils import run_bass_kernel_spmd

    caps, in_maps, books = _marshal(**inputs)
    nc, calls, boff = _get_nc(caps)
    r = run_bass_kernel_spmd(
        nc, in_maps, core_ids=list(range(N_CORES)), trace=trace, **run_kwargs
    )
    out = _unmarshal(r.results, books, caps, len(inputs["nodes_from_to"]))
    return out, r


def kernel(**inputs) -> np.ndarray:
    out, _ = _run(inputs, trace=False)
    return out


# revision 93
# speedup vs baseline: 1.0014x; 1.0014x over previous
"""Trainium2 Bass kernel for nn_BetweenClusterFC.

Computes out[e] = (emb_1[f[e]] @ W1 + b1) . (emb_2[t[e]] @ W2 + b2)
for E = 1.6M edges over N = 100k nodes, D_IN = 256, D_OUT = 128.

Strategy (8 NeuronCores, SPMD, full inputs in / full output out):
  - Nodes are split into 8 blocks of 12500.  Edges are assigned to cores by a
    (from-block-group, to-block-group) 4x2 rectangle: core c=(a,b) handles
    edges with from-node in blocks [4a..4a+3] and to-node in blocks
    [2b..2b+1].  Uniform (~200k edges/core); each core needs projections for
    4 from-blocks + 2 to-blocks (75k nodes).
  - All streamed data is bf16: the host pre-transposes and casts the
    embedding shards; the PE projects p = emb @ W (+ bias folded in as a
    K=1 ones x bias matmul), the Act engine moves psum -> sbuf in bf16 and
    issues the p-table writes (1KB-contiguous via a (p t)-permuted row
    order), keeping the DVE entirely free for the edge phase.
  - Edges are bucketed host-side by (local from-block, local to-block); per
    bucket both endpoint rows are fetched with SWDGE dma_gather calls (1024
    idxs/call HW limit, two DMA queues, rows moved as packed f32 pairs)
    using int16 permuted-local indices.  Calls land in contiguous 4096-edge
    "items" (two full gather pairs sharing one of 5 rotating super-slots);
    the DVE runs a software-pipelined bf16 multiply (2x mode) + bf16
    tree-halving + short f32 reduce per item into a single SBUF-resident
    result tile, written out in two halves.
  - Bucket capacities are sized from the actual per-core edge counts (max
    across cores, 128-aligned), so the Bass program is built per problem
    instance (inspector-executor style) and cached.  Bucket 0 is ordered
    early-prefix-rows first so its first gather pairs unblock after 38 of
    150 projection groups; the idle pre-gather Pool engine prefetches the
    idx tables and part of the embT stream; the DVE covers half the early
    psum->sbuf copies; p-table writes go out as two-group pair DMAs to
    amortize the per-DMA descriptor-generation floor.
  - The host applies the inverse edge permutation to assemble the output.

Written in raw Bass (explicit semaphores) - the Tile layer's generated sync
exceeds this toolchain's per-instruction wait-slot limits.
"""

import contextlib

import numpy as np

import concourse.bass as bass
import concourse.mybir as mybir

# ---------------------------------------------------------------- constants
N_NODES = 100_000
D_IN = 256
D_OUT = 128
N_EDGES = 1_600_000
N_CORES = 8

NB = 12_500          # nodes per block
NBP = 12_800         # padded block rows (25 * 512) - multiple of 512
NFB = 4              # from-blocks per core
NTB = 2              # to-blocks per core
NBUCKET = NFB * NTB  # 8 buckets per core

P1_ROWS = NFB * NBP  # 51200
P2_ROWS = NTB * NBP  # 25600

TILES1 = P1_ROWS // 128    # 400 node-tiles, table 1
TILES2 = P2_ROWS // 128    # 200 node-tiles, table 2
GROUPS1 = TILES1 // 4      # 100 psum groups
GROUPS2 = TILES2 // 4      # 50
NGROUP = GROUPS1 + GROUPS2  # 150
CHUNK_T = 20               # node-tiles per embT load chunk (= 5 groups)
NCH1 = TILES1 // CHUNK_T   # 20 chunks
NCH2 = TILES2 // CHUNK_T   # 10
NCHUNK = NCH1 + NCH2       # 30
EMB_COLS = CHUNK_T * 128   # 2560

MAX_CALL = 1024            # dma_gather idxs per call (HW limit; ring 4096/queue)
HROWS = 6_656              # p1-block-0 prefix rows (13 groups; pair-even gate)
PH1 = 52                   # DVE assists psum->pv copies for odd groups < PH1

F32 = mybir.dt.float32
BF16 = mybir.dt.bfloat16
I16 = mybir.dt.int16
AX = mybir.AxisListType


# Projection block order: interleave p2/p1 blocks so gather buckets unblock
# as early as possible (bucket (fi,ti) needs p1 block fi + p2 block ti):
# (0,0)@50 groups, (0,1)@75, (1,*)@100, (2,*)@125, (3,*)@150.
BSEQ = [(1, 0), (0, 0), (1, 1), (0, 1), (0, 2), (0, 3)]
GPB = NBP // 512   # 25 groups per block
CPB = GPB // 5     # 5 chunks per block (CHUNK_T = 4 groups... 20 tiles)
GSEQ = []
CSEQ = []
for tab, blk in BSEQ:
    g0 = blk * GPB + (GROUPS1 if tab == 1 else 0)
    GSEQ += list(range(g0, g0 + GPB))
    c0 = blk * 5 + (NCH1 if tab == 1 else 0)
    CSEQ += list(range(c0, c0 + 5))
CPOS = {cid: q for q, cid in enumerate(CSEQ)}
_BPOS = {tb: i for i, tb in enumerate(BSEQ)}
# bucket (fi,ti) ready after this many groups in GSEQ order
BK_READY_Q = [GPB * (1 + max(_BPOS[(0, fi)], _BPOS[(1, ti)]))
              for fi in range(NFB) for ti in range(NTB)]


def _chunk_of_tile(tg):
    """global tile index -> (global chunk id, local col0 within chunk)."""
    if tg < TILES1:
        c = tg // CHUNK_T
        return c, (tg % CHUNK_T) * 128
    t2 = tg - TILES1
    c = NCH1 + t2 // CHUNK_T
    return c, (t2 % CHUNK_T) * 128


def _chunk_src(c):
    """global chunk id -> (table, col0)."""
    if c < NCH1:
        return 0, c * EMB_COLS
    return 1, (c - NCH1) * EMB_COLS


def _plan_calls(caps):
    """caps: per-bucket slot capacities (multiples of 128).
    Returns a list of gather-call PAIRS (bucket, n0, n1, idx_col0, slot_col0):
    two <=1024-idx gather calls whose outputs land adjacently so the DVE can
    process 2048 edges per op chain (n1 == 0 for a lone remainder call).
    Plus (idx_cols_total, slot_cols_total, per-bucket slot offsets)."""
    pairs = []
    icol = 0
    scol = 0
    boff = []
    for bk, cap in enumerate(caps):
        boff.append(scol)
        left = cap
        while left > 0:
            n0 = min(MAX_CALL, left)
            n1 = min(MAX_CALL, left - n0)
            pairs.append((bk, n0, n1, icol, scol))
            icol += (n0 + n1) // 16
            scol += (n0 + n1) // 128
            left -= n0 + n1
    return pairs, icol, scol, boff


# ---------------------------------------------------------------- device code
def build_bass(caps, ep0):
    """caps: per-bucket capacities (edge slots, multiples of 128), shared by
    all cores (max over cores).  ep0: bucket-0 pairs gathering only the
    HROWS-row prefix of p1 block 0 (host orders those edges first)."""
    calls, idx_cols, tot_slots, boff = _plan_calls(caps)
    ncall = len(calls)

    nc = bass.Bass(dynamic_dma_scratch_size=32768, num_swdge_queues=2)

    e1t = nc.dram_tensor("e1t", [D_IN, P1_ROWS], BF16, kind="ExternalInput")
    e2t = nc.dram_tensor("e2t", [D_IN, P2_ROWS], BF16, kind="ExternalInput")
    w12 = nc.dram_tensor("w12", [D_IN, 2 * D_OUT], BF16, kind="ExternalInput")
    bo3 = nc.dram_tensor("bo3", [3, D_OUT], BF16, kind="ExternalInput")
    idxa = nc.dram_tensor("idxa", [128, idx_cols], I16, kind="ExternalInput")
    idxb = nc.dram_tensor("idxb", [128, idx_cols], I16, kind="ExternalInput")
    res = nc.dram_tensor("res", [128, tot_slots], F32, kind="ExternalOutput")

    p1d = nc.dram_tensor("p1d", [P1_ROWS, D_OUT], BF16, kind="Internal")
    p2d = nc.dram_tensor("p2d", [P2_ROWS, D_OUT], BF16, kind="Internal")
    pdst = (p1d, p2d)

    st = contextlib.ExitStack()
    with st:
        sb = lambda nm, shape, dt: st.enter_context(nc.sbuf_tensor(nm, shape, dt))
        sem = lambda nm: st.enter_context(nc.semaphore(name=nm))

        # wc columns: [w1_k0 | w2_k0 | w1_k1 | w2_k1] (128 each)
        wc = sb("wc", [128, 512], BF16)
        bt = [sb(f"bt{i}", [1, 128], BF16) for i in range(3)]  # b1, b2, ones
        idxt = (sb("idxta", [128, idx_cols], I16), sb("idxtb", [128, idx_cols], I16))
        et = [[sb(f"et_{p}_{k}", [128, EMB_COLS], BF16) for k in range(2)]
              for p in range(3)]  # [buffer cq%3][k-half]
        pvt = sb("pvt", [128, 4 * 512], BF16)
        pv = [pvt[:, i * 512:(i + 1) * 512] for i in range(4)]
        ps = [st.enter_context(nc.psum_tensor(f"ps{i}", [128, 512], F32))
              for i in range(4)]
        # Gather pairs are merged into "items" of up to two full pairs (4096
        # edges) sharing one contiguous super-slot, so each DVE chain op's
        # fixed cost is amortized over twice the edges.  4 rotating slots.
        NBUF = 5
        ITEM_COLS = 4 * MAX_CALL
        atb = sb("atb", [128, NBUF * ITEM_COLS], BF16)
        btgb = sb("btgb", [128, NBUF * ITEM_COLS], BF16)
        rt = sb("rt", [128, tot_slots], F32)

        s_cl = sem("s_cl")               # w const loads (2 dmas -> 32)
        s_clb = sem("s_clb")             # bias/ones const loads (3 -> 48)
        s_cli = sem("s_cli")             # idx table loads (2 dmas -> 32)
        s_load = tuple(sem(f"s_load{i}") for i in range(3))  # SP embT, by cq%3
        s_loadp = sem("s_loadp")         # Pool embT loads
        s_mm = sem("s_mm")               # matmuls (+1 each; 3 per tile)
        s_cp = sem("s_cp")               # act copies (+1 each)
        s_cpd = sem("s_cpd")             # dve copies (+1 each)
        s_pw = tuple(sem(f"s_pw{i}") for i in range(2))  # pair writes, by j%2
        s_pwx = sem("s_pwx")             # first halves of crossing pairs
        # gather-completion sems by item%NBUF per queue: item w+NBUF's gathers
        # are gated on s_red >= w+1, so updates of each sem are causally
        # ordered.
        s_ga = tuple(sem(f"s_ga{i}") for i in range(NBUF))  # queue-0, +16/call
        s_gb = tuple(sem(f"s_gb{i}") for i in range(NBUF))  # queue-1, +16/call
        s_st = tuple(sem(f"s_st{i}") for i in range(4))  # DVE dot stages 0-3
        s_red = sem("s_red")             # final reduces (+1 per item)
        s_out = sem("s_out")             # res dma

        CONSTS = 2 * 16

        # items: (pair list, slot columns per pair member, total slots, scol)
        items = []
        _p = 0
        while _p < ncall:
            bk0, n0, n1, icol0, scol0 = calls[_p]
            if (n0 + n1 == 2 * MAX_CALL and _p + 1 < ncall
                    and calls[_p + 1][0] == bk0):
                items.append(([_p, _p + 1], scol0))
                _p += 2
            else:
                items.append(([_p], scol0))
                _p += 1
        nitem = len(items)
        item_slots = [sum((calls[p][1] + calls[p][2]) // 128 for p in ps_)
                      for ps_, _ in items]

        # per-sem cumulative gather counts per item
        _sem_cnt = [0] * NBUF
        gwait = []
        for _w, (_ps, _sc) in enumerate(items):
            _sem_cnt[_w % NBUF] += 16 * sum(
                2 if calls[p][2] else 1 for p in _ps)
            gwait.append(_sem_cnt[_w % NBUF])

        # During the pre-gather phase the DVE is mostly idle: it takes the odd
        # groups' psum->pv copies for the first PH1 groups.
        dve_copy = [q < PH1 and q % 2 == 1 for q in range(NGROUP)]
        cp_cnt = []  # cumulative (act, dve) copy counts through group q
        _na = _ndv = 0
        for q in range(NGROUP):
            if dve_copy[q]:
                _ndv += 1
            else:
                _na += 1
            cp_cnt.append((_na, _ndv))

        def _copy_wait(eng, q):
            """wait until the copy of group q is complete."""
            if dve_copy[q]:
                eng.wait_ge(s_cpd, cp_cnt[q][1])
            else:
                eng.wait_ge(s_cp, cp_cnt[q][0])

        block = st.enter_context(nc.Block())

        # Chunks 1,3,5,7,9 are loaded by the Pool engine during its pre-gather
        # idle window (own sem s_loadp); SP loads the rest (s_load by parity).
        POOL_CHUNKS = (4, 6, 8)
        chunk_wait = {}
        _cnt = {0: 0, 1: 0, 2: 0, 'p': 0}
        for _cq in range(NCHUNK):
            key = 'p' if _cq in POOL_CHUNKS else _cq % 3
            _cnt[key] += 32
            chunk_wait[_cq] = (key, _cnt[key])

        # p-writes happen per PAIR of groups (2j, 2j+1): pv slots are adjacent
        # so one DMA covers both, halving Act's per-write descriptor-gen
        # floor.  Pairs whose two groups land in different projection blocks
        # ("crossing": non-contiguous table rows) are written as two singles,
        # the first tracked by s_pwx.
        NPAIR_W = NGROUP // 2
        CROSSING = frozenset(j for j in range(NPAIR_W)
                             if GSEQ[2 * j + 1] != GSEQ[2 * j] + 1)
        _xrank = {j: i + 1 for i, j in enumerate(sorted(CROSSING))}

        def _one_write(eng, q, s_sem, cnt):
            g = GSEQ[q]
            tab = 0 if g < GROUPS1 else 1
            r0 = g * 512 if tab == 0 else (g - GROUPS1) * 512
            # (p t)-permuted write: table row r0 + p*4 + t <- node r0 + t*128 + p
            eng.dma_start(
                out=pdst[tab][r0:r0 + 512, :].rearrange("(p t) d -> p t d", p=128),
                in_=pv[q % 4][:].rearrange("p (t d) -> p t d", d=128),
            ).then_inc(s_sem, cnt)

        def p_write_pair(eng, j):
            if j in CROSSING:
                _one_write(eng, 2 * j, s_pwx, 16)
                _one_write(eng, 2 * j + 1, s_pw[j % 2], 16)
                return
            q0 = 2 * j
            g = GSEQ[q0]
            tab = 0 if g < GROUPS1 else 1
            r0 = g * 512 if tab == 0 else (g - GROUPS1) * 512
            s0 = q0 % 4
            eng.dma_start(
                out=pdst[tab][r0:r0 + 1024, :]
                    .rearrange("(g p t) d -> p g t d", p=128, t=4),
                in_=pvt[:, s0 * 512:(s0 + 2) * 512]
                    .rearrange("p (g t d) -> p g t d", t=4, d=128),
            ).then_inc(s_pw[j % 2], 16)

        def wait_pairs_through(eng, nq):
            """wait until all p-writes for groups < nq are complete (nq is
            rounded up to a pair boundary)."""
            npair = (nq + 1) // 2
            for r in range(2):
                eng.wait_ge(s_pw[r], 16 * len(range(r, npair, 2)))
            nx = sum(1 for j in CROSSING if j < npair)
            if nx:
                eng.wait_ge(s_pwx, 16 * nx)

        def wait_pv_drained(eng, q):
            """wait until pv slot q%4 (last used by group q-4) is rewritable."""
            p4 = (q - 4) // 2
            eng.wait_ge(s_pw[p4 % 2], 16 * (p4 // 2 + 1))
            if p4 in CROSSING:
                eng.wait_ge(s_pwx, 16 * _xrank[p4])

        def load_chunk(eng, cq):
            if cq >= 3:
                # buffer cq%3 previously held chunk cq-3; wait consumed
                eng.wait_ge(s_mm, 3 * CHUNK_T * (cq - 2))
            tab, col0 = _chunk_src(CSEQ[cq])
            src = e1t if tab == 0 else e2t
            par = cq % 3
            s_ld = s_loadp if cq in POOL_CHUNKS else s_load[cq % 3]
            eng.dma_start(out=et[par][0][:],
                          in_=src[0:128, col0:col0 + EMB_COLS]).then_inc(s_ld, 16)
            eng.dma_start(out=et[par][1][:],
                          in_=src[128:256, col0:col0 + EMB_COLS]).then_inc(s_ld, 16)

        # ------------------------------------------------ SP: embT + p-writes
        @block.sync
        def _(sync):
            for k in range(2):
                sync.dma_start(out=wc[:, k * 256:(k + 1) * 256],
                               in_=w12[k * 128:(k + 1) * 128, :]).then_inc(s_cl, 16)
            load_chunk(sync, 0)
            for i in range(3):
                sync.dma_start(out=bt[i][:], in_=bo3[i:i + 1, :]).then_inc(s_clb, 16)
            load_chunk(sync, 1)
            next_cq = 2
            for q in range(NGROUP):
                # look ahead: issue loads for chunks starting within 5 groups
                while next_cq < NCHUNK and next_cq * CHUNK_T <= (q + 5) * 4 + 3:
                    if next_cq not in POOL_CHUNKS:
                        load_chunk(sync, next_cq)
                    next_cq += 1


        # ------------------------------------------------ PE: projections
        @block.tensor
        def _(tensor):
            tensor.wait_ge(s_cl, CONSTS)
            for q, g in enumerate(GSEQ):
                tab = 0 if g < GROUPS1 else 1
                bc = bt[tab][:]
                for j in range(4):
                    tq = q * 4 + j
                    cid, col0 = _chunk_of_tile(g * 4 + j)
                    cq = CPOS[cid]
                    if tq == cq * CHUNK_T:  # first processed tile of chunk
                        key, cntv = chunk_wait[cq]
                        tensor.wait_ge(s_loadp if key == 'p' else s_load[key], cntv)
                    if j == 0 and q >= 4:
                        _copy_wait(tensor, q - 4)  # psum bank q%4 free
                    out = ps[q % 4][:, j * 128:(j + 1) * 128]
                    tensor.matmul(out=out, lhsT=et[cq % 3][0][:, col0:col0 + 128],
                                  rhs=wc[:, tab * 128:tab * 128 + 128],
                                  start=True, stop=False).then_inc(s_mm, 1)
                    tensor.matmul(out=out, lhsT=et[cq % 3][1][:, col0:col0 + 128],
                                  rhs=wc[:, 256 + tab * 128:256 + tab * 128 + 128],
                                  start=False, stop=False).then_inc(s_mm, 1)
                    if tq == 0:
                        tensor.wait_ge(s_clb, 48)
                    tensor.matmul(out=out, lhsT=bt[2][:], rhs=bc,
                                  start=False, stop=True).then_inc(s_mm, 1)

        # ------------------------------------------------ Act: psum->sbuf + DMAs
        @block.scalar
        def _(scalar):
            scalar.wait_ge(s_cl, CONSTS)
            # the pair write of groups (q-2, q-1) is issued after the copy of
            # group q so the copy-completion sems are already visible.
            for q, g in enumerate(GSEQ):
                if not dve_copy[q]:
                    scalar.wait_ge(s_mm, 12 * q + 12)
                    if q >= 4:
                        wait_pv_drained(scalar, q)
                    scalar.copy(out=pv[q % 4][:], in_=ps[q % 4][:]).then_inc(s_cp, 1)
                if q >= 2 and q % 2 == 0:
                    _copy_wait(scalar, q - 2)
                    _copy_wait(scalar, q - 1)
                    p_write_pair(scalar, (q - 2) // 2)
            _copy_wait(scalar, NGROUP - 2)
            _copy_wait(scalar, NGROUP - 1)
            p_write_pair(scalar, NPAIR_W - 1)
            # write results in thirds so only the last sliver is exposed
            cut1, cut2 = nitem // 3, (2 * nitem) // 3
            s1, s2 = items[cut1][1], items[cut2][1]
            scalar.wait_ge(s_red, cut1)
            scalar.dma_start(out=res[:, :s1], in_=rt[:, :s1]).then_inc(s_out, 16)
            scalar.wait_ge(s_red, cut2)
            scalar.dma_start(out=res[:, s1:s2], in_=rt[:, s1:s2]).then_inc(s_out, 16)
            scalar.wait_ge(s_red, nitem)
            scalar.dma_start(out=res[:, s2:], in_=rt[:, s2:]).then_inc(s_out, 16)
            scalar.wait_ge(s_out, 48)

        # ------------------------------------------------ Pool: gathers
        @block.gpsimd
        def _(gpsimd):
            # the Pool engine is idle until the first bucket gate (~40us): it
            # loads the consts, every other early embT chunk, and its own idx
            # tables, so SP's chunk stream (which feeds the PE) never blocks.
            gpsimd.dma_start(out=idxt[0][:], in_=idxa[:]).then_inc(s_cli, 16)
            gpsimd.dma_start(out=idxt[1][:], in_=idxb[:]).then_inc(s_cli, 16)
            for i, cq in enumerate(POOL_CHUNKS):
                if i:
                    gpsimd.wait_ge(s_loadp, 32 * i)  # order s_loadp updates
                load_chunk(gpsimd, cq)
            from concourse import library_config
            gpsimd.load_library(library_config.mlp)
            sizes = ({c[1] for c in calls} | {c[2] for c in calls}) - {0}
            regs = {n: gpsimd.to_reg(n) for n in sorted(sizes)}
            gpsimd.wait_ge(s_cl, CONSTS)
            gpsimd.wait_ge(s_cli, 32)

            # first ep0 pairs of bucket 0 reference only p1 rows < HROWS (the
            # host orders bucket-0 edges early-rows-first), so they unblock
            # after 25 + HROWS/512 projected groups instead of 50.
            gate_req = []
            for p, (bk, n0, n1, icol, scol) in enumerate(calls):
                if bk == 0 and p < ep0:
                    gate_req.append(GPB + HROWS // 512)
                else:
                    gate_req.append(BK_READY_Q[bk])

            cur_gate = -1
            for w, (ps_, scol_w) in enumerate(items):
                if w >= NBUF:
                    gpsimd.wait_ge(s_red, w - NBUF + 1)  # slot w%NBUF consumed
                base = (w % NBUF) * ITEM_COLS
                nh = 0
                for p in ps_:
                    bk, n0, n1, icol, scol = calls[p]
                    fi, ti = bk // NTB, bk % NTB
                    if gate_req[p] > cur_gate:
                        cur_gate = gate_req[p]
                        wait_pairs_through(gpsimd, cur_gate)
                    rows1 = HROWS if (bk == 0 and p < ep0) else NBP
                    # rows move as 64 packed f32 (= 128 bf16): same bytes and
                    # descriptors, half the modeled element count.
                    for n_i, n in enumerate((n0, n1)):
                        if n == 0:
                            continue
                        S = n // 128
                        c0 = icol + (nh % (2 * MAX_CALL)) // 16
                        gpsimd.dma_gather(
                            out_ap=atb[:, base + nh:base + nh + S * 128]
                                .bitcast(F32).rearrange("p (s d) -> p s d", d=64),
                            in_ap=p1d[fi * NBP:fi * NBP + rows1, :].bitcast(F32),
                            idxs_ap=idxt[0][:, c0:c0 + n // 16],
                            num_idxs=n, num_idxs_reg=regs[n],
                            elem_size=D_OUT // 2, queue_num=0,
                        ).then_inc(s_ga[w % NBUF], 16)
                        gpsimd.dma_gather(
                            out_ap=btgb[:, base + nh:base + nh + S * 128]
                                .bitcast(F32).rearrange("p (s d) -> p s d", d=64),
                            in_ap=p2d[ti * NBP:(ti + 1) * NBP, :].bitcast(F32),
                            idxs_ap=idxt[1][:, c0:c0 + n // 16],
                            num_idxs=n, num_idxs_reg=regs[n],
                            elem_size=D_OUT // 2, queue_num=1,
                        ).then_inc(s_gb[w % NBUF], 16)
                        nh += S * 128

        # ------------------------------------------------ DVE: dot products
        # bf16 multiply (2x mode), tree-halve in bf16 (2x) down to 16 partials
        # per slot, then one short f32 reduce - one chain per gather pair
        # (up to 2048 edges).  The 5-stage chain is software-pipelined across
        # pairs (stage s of pair p in round p+s) so every intra-chain
        # semaphore is already visible when its wait issues.
        @block.vector
        def _(vector):
            # phase-1 assist: odd-group psum->pv copies while gathers are gated
            for q in range(NGROUP):
                if not dve_copy[q]:
                    continue
                vector.wait_ge(s_mm, 12 * q + 12)
                if q >= 4:
                    wait_pv_drained(vector, q)
                vector.tensor_copy(out=pv[q % 4][:], in_=ps[q % 4][:]).then_inc(s_cpd, 1)

            def stage(s, w):
                S = item_slots[w]
                scol_w = items[w][1]
                base = (w % NBUF) * ITEM_COLS
                a3 = atb[:, base:base + S * 128]
                av = a3.rearrange("p (s d) -> p s d", d=128)
                if s == 0:
                    vector.wait_ge(s_ga[w % NBUF], gwait[w])
                    vector.wait_ge(s_gb[w % NBUF], gwait[w])
                    vector.tensor_mul(out=a3, in0=a3,
                                      in1=btgb[:, base:base + S * 128]
                                      ).then_inc(s_st[0], 1)
                elif s in (1, 2, 3):
                    h = 128 >> s  # 64, 32, 16
                    vector.wait_ge(s_st[s - 1], w + 1)
                    vector.tensor_add(out=av[:, :, 0:h], in0=av[:, :, 0:h],
                                      in1=av[:, :, h:2 * h]).then_inc(s_st[s], 1)
                else:
                    vector.wait_ge(s_st[3], w + 1)
                    vector.reduce_sum(out=rt[:, scol_w:scol_w + S],
                                      in_=av[:, :, 0:16], axis=AX.X).then_inc(s_red, 1)

            for r in range(nitem + 4):
                for s in range(4, -1, -1):
                    w = r - s
                    if 0 <= w < nitem:
                        stage(s, w)

    return nc, calls, boff


_NC_CACHE: dict = {}


def _get_nc(caps):
    caps, ep0 = caps
    key = (tuple(caps), ep0)
    if key not in _NC_CACHE:
        nc, calls, boff = build_bass(caps, ep0)
        from concourse.library_overlay import lower_extended_insts
        lower_extended_insts(nc)
        _NC_CACHE[key] = (nc, calls, boff)
    return _NC_CACHE[key]


# ---------------------------------------------------------------- host side
def _perm_local(n):
    """block-local node id -> permuted table row (within block).
    Table row g*512 + p*4 + t holds node g*512 + t*128 + p."""
    g, u = np.divmod(n, 512)
    t, p = np.divmod(u, 128)
    return g * 512 + p * 4 + t


def _marshal(emb_1, emb_2, nodes_from_to, W1, b1, W2, b2):
    """Shard/bucket inputs per core.  Returns (caps, in_maps, books)."""
    import ml_dtypes
    bf16 = ml_dtypes.bfloat16

    f = np.asarray(nodes_from_to[:, 0], dtype=np.int64)
    t = np.asarray(nodes_from_to[:, 1], dtype=np.int64)
    emb_1 = np.asarray(emb_1, dtype=np.float32)
    emb_2 = np.asarray(emb_2, dtype=np.float32)
    w12 = np.concatenate(
        [np.asarray(W1, dtype=np.float32), np.asarray(W2, dtype=np.float32)],
        axis=1).astype(bf16)
    bo3 = np.stack([
        np.asarray(b1, dtype=np.float32).reshape(-1),
        np.asarray(b2, dtype=np.float32).reshape(-1),
        np.ones(D_OUT, np.float32),
    ]).astype(bf16)

    core = (f // (NFB * NB)) * 4 + t // (NTB * NB)
    order0 = np.argsort(core, kind="stable")
    ccnt = np.bincount(core, minlength=N_CORES)
    coff = np.concatenate([[0], np.cumsum(ccnt)])

    percore = []
    early_cnts = []
    all_cnts = np.zeros((N_CORES, NBUCKET), np.int64)
    for c in range(N_CORES):
        a, b = c // 4, c % 4
        sel = order0[coff[c]:coff[c + 1]]
        fc, tcv = f[sel], t[sel]
        fi = fc // NB - NFB * a
        ti = tcv // NB - NTB * b
        fl = _perm_local(fc % NB).astype(np.int16)
        tl = _perm_local(tcv % NB).astype(np.int16)
        bk = fi * NTB + ti
        o2 = np.argsort(bk, kind="stable")
        sel2, fl2, tl2 = sel[o2], fl[o2], tl[o2]
        cnts = np.bincount(bk, minlength=NBUCKET)
        all_cnts[c] = cnts
        # bucket 0: early (prefix-row) edges first, enabling the half-gate
        n0 = cnts[0]
        e0 = fl2[:n0] < HROWS
        o3 = np.argsort(~e0, kind="stable")
        sel2[:n0], fl2[:n0], tl2[:n0] = sel2[:n0][o3], fl2[:n0][o3], tl2[:n0][o3]
        early_cnts.append(int(e0.sum()))
        percore.append((a, b, sel2, fl2, tl2, cnts))

    caps = [int(-(-all_cnts[:, k].max() // 128) * 128) for k in range(NBUCKET)]
    ep0 = min(early_cnts) // (2 * MAX_CALL)
    calls, idx_cols, tot_slots, boff = _plan_calls(caps)

    in_maps, books = [], []
    for c in range(N_CORES):
        a, b, sel2, fl2, tl2, cnts = percore[c]
        pos = np.concatenate([[0], np.cumsum(cnts)])

        slots_a = np.zeros((NBUCKET, max(caps)), np.int16)
        slots_b = np.zeros((NBUCKET, max(caps)), np.int16)
        for k in range(NBUCKET):
            slots_a[k, :cnts[k]] = fl2[pos[k]:pos[k + 1]]
            slots_b[k, :cnts[k]] = tl2[pos[k]:pos[k + 1]]
        # wrap by 16: idx i of a bucket at (partition i%16, col i//16),
        # replicated across the 8 groups of 16 partitions
        wa_cols = []
        wb_cols = []
        for k in range(NBUCKET):
            cap = caps[k]
            wa_cols.append(slots_a[k, :cap].reshape(cap // 16, 16).T)
            wb_cols.append(slots_b[k, :cap].reshape(cap // 16, 16).T)
        idxa = np.tile(np.concatenate(wa_cols, axis=1), (8, 1))
        idxb = np.tile(np.concatenate(wb_cols, axis=1), (8, 1))

        e1t = np.zeros((D_IN, P1_ROWS), bf16)
        for i in range(NFB):
            blk = emb_1[(NFB * a + i) * NB:(NFB * a + i + 1) * NB]
            e1t[:, i * NBP:i * NBP + NB] = blk.T.astype(bf16)
        e2t = np.zeros((D_IN, P2_ROWS), bf16)
        for i in range(NTB):
            blk = emb_2[(NTB * b + i) * NB:(NTB * b + i + 1) * NB]
            e2t[:, i * NBP:i * NBP + NB] = blk.T.astype(bf16)

        in_maps.append({
            "e1t": e1t, "e2t": e2t, "w12": w12, "bo3": bo3,
            "idxa": np.ascontiguousarray(idxa),
            "idxb": np.ascontiguousarray(idxb),
        })
        books.append((sel2, cnts, pos))
    return (caps, ep0), in_maps, books


def _unmarshal(results, books, caps, n_edges):
    calls, idx_cols, tot_slots, boff = _plan_calls(caps[0])
    out = np.empty(n_edges, np.float32)
    for c in range(N_CORES):
        sel2, cnts, pos = books[c]
        r = results[c]["res"]  # [128, tot_slots]
        for k in range(NBUCKET):
            if cnts[k] == 0:
                continue
            s0 = boff[k]
            nslots = caps[0][k] // 128
            stream = r[:, s0:s0 + nslots].T.reshape(-1)
            out[sel2[pos[k]:pos[k + 1]]] = stream[:cnts[k]]
    return out


def _run(inputs, trace=False, **run_kwargs):
    from concourse.bass_utils import run_bass_kernel_spmd

    caps, in_maps, books = _marshal(**inputs)
    nc, calls, boff = _get_nc(caps)
    r = run_bass_kernel_spmd(
        nc, in_maps, core_ids=list(range(N_CORES)), trace=trace, **run_kwargs
    )
    out = _unmarshal(r.results, books, caps, len(inputs["nodes_from_to"]))
    return out, r


def kernel(**inputs) -> np.ndarray:
    out, _ = _run(inputs, trace=False)
    return out


# revision 95
# speedup vs baseline: 1.0031x; 1.0017x over previous
"""Trainium2 Bass kernel for nn_BetweenClusterFC.

Computes out[e] = (emb_1[f[e]] @ W1 + b1) . (emb_2[t[e]] @ W2 + b2)
for E = 1.6M edges over N = 100k nodes, D_IN = 256, D_OUT = 128.

Strategy (8 NeuronCores, SPMD, full inputs in / full output out):
  - Nodes are split into 8 blocks of 12500.  Edges are assigned to cores by a
    (from-block-group, to-block-group) 4x2 rectangle: core c=(a,b) handles
    edges with from-node in blocks [4a..4a+3] and to-node in blocks
    [2b..2b+1].  Uniform (~200k edges/core); each core needs projections for
    4 from-blocks + 2 to-blocks (75k nodes).
  - All streamed data is bf16: the host pre-transposes and casts the
    embedding shards; the PE projects p = emb @ W (+ bias folded in as a
    K=1 ones x bias matmul), the Act engine moves psum -> sbuf in bf16 and
    issues the p-table writes (1KB-contiguous via a (p t)-permuted row
    order), keeping the DVE entirely free for the edge phase.
  - Edges are bucketed host-side by (local from-block, local to-block); per
    bucket both endpoint rows are fetched with SWDGE dma_gather calls (1024
    idxs/call HW limit, two DMA queues, rows moved as packed f32 pairs)
    using int16 permuted-local indices.  Calls land in contiguous 4096-edge
    "items" (two full gather pairs sharing one of 5 rotating super-slots);
    the DVE runs a software-pipelined bf16 multiply (2x mode) + bf16
    tree-halving + short f32 reduce per item into a single SBUF-resident
    result tile, written out in thirds as the reduces complete.
  - Bucket capacities are sized from the actual per-core edge counts (max
    across cores, 128-aligned), so the Bass program is built per problem
    instance (inspector-executor style) and cached.  Bucket 0 is ordered
    early-prefix-rows first so its first gather pairs unblock after 38 of
    150 projection groups; the idle pre-gather Pool engine prefetches the
    idx tables and part of the embT stream; the DVE covers half the early
    psum->sbuf copies; p-table writes go out as two-group pair DMAs to
    amortize the per-DMA descriptor-generation floor.
  - The host applies the inverse edge permutation to assemble the output.

Written in raw Bass (explicit semaphores) - the Tile layer's generated sync
exceeds this toolchain's per-instruction wait-slot limits.
"""

import contextlib

import numpy as np

import concourse.bass as bass
import concourse.mybir as mybir

# ---------------------------------------------------------------- constants
N_NODES = 100_000
D_IN = 256
D_OUT = 128
N_EDGES = 1_600_000
N_CORES = 8

NB = 12_500          # nodes per block
NBP = 12_800         # padded block rows (25 * 512) - multiple of 512
NFB = 4              # from-blocks per core
NTB = 2              # to-blocks per core
NBUCKET = NFB * NTB  # 8 buckets per core

P1_ROWS = NFB * NBP  # 51200
P2_ROWS = NTB * NBP  # 25600

TILES1 = P1_ROWS // 128    # 400 node-tiles, table 1
TILES2 = P2_ROWS // 128    # 200 node-tiles, table 2
GROUPS1 = TILES1 // 4      # 100 psum groups
GROUPS2 = TILES2 // 4      # 50
NGROUP = GROUPS1 + GROUPS2  # 150
CHUNK_T = 20               # node-tiles per embT load chunk (= 5 groups)
NCH1 = TILES1 // CHUNK_T   # 20 chunks
NCH2 = TILES2 // CHUNK_T   # 10
NCHUNK = NCH1 + NCH2       # 30
EMB_COLS = CHUNK_T * 128   # 2560

MAX_CALL = 1024            # dma_gather idxs per call (HW limit; ring 4096/queue)
HROWS = 6_656              # p1-block-0 prefix rows (13 groups; pair-even gate)
PH1 = 54                   # DVE assists psum->pv copies for odd groups < PH1

F32 = mybir.dt.float32
BF16 = mybir.dt.bfloat16
I16 = mybir.dt.int16
AX = mybir.AxisListType


# Projection block order: interleave p2/p1 blocks so gather buckets unblock
# as early as possible (bucket (fi,ti) needs p1 block fi + p2 block ti):
# (0,0)@50 groups, (0,1)@75, (1,*)@100, (2,*)@125, (3,*)@150.
BSEQ = [(1, 0), (0, 0), (1, 1), (0, 1), (0, 2), (0, 3)]
GPB = NBP // 512   # 25 groups per block
CPB = GPB // 5     # 5 chunks per block (CHUNK_T = 4 groups... 20 tiles)
GSEQ = []
CSEQ = []
for tab, blk in BSEQ:
    g0 = blk * GPB + (GROUPS1 if tab == 1 else 0)
    GSEQ += list(range(g0, g0 + GPB))
    c0 = blk * 5 + (NCH1 if tab == 1 else 0)
    CSEQ += list(range(c0, c0 + 5))
CPOS = {cid: q for q, cid in enumerate(CSEQ)}
_BPOS = {tb: i for i, tb in enumerate(BSEQ)}
# bucket (fi,ti) ready after this many groups in GSEQ order
BK_READY_Q = [GPB * (1 + max(_BPOS[(0, fi)], _BPOS[(1, ti)]))
              for fi in range(NFB) for ti in range(NTB)]


def _chunk_of_tile(tg):
    """global tile index -> (global chunk id, local col0 within chunk)."""
    if tg < TILES1:
        c = tg // CHUNK_T
        return c, (tg % CHUNK_T) * 128
    t2 = tg - TILES1
    c = NCH1 + t2 // CHUNK_T
    return c, (t2 % CHUNK_T) * 128


def _chunk_src(c):
    """global chunk id -> (table, col0)."""
    if c < NCH1:
        return 0, c * EMB_COLS
    return 1, (c - NCH1) * EMB_COLS


def _plan_calls(caps):
    """caps: per-bucket slot capacities (multiples of 128).
    Returns a list of gather-call PAIRS (bucket, n0, n1, idx_col0, slot_col0):
    two <=1024-idx gather calls whose outputs land adjacently so the DVE can
    process 2048 edges per op chain (n1 == 0 for a lone remainder call).
    Plus (idx_cols_total, slot_cols_total, per-bucket slot offsets)."""
    pairs = []
    icol = 0
    scol = 0
    boff = []
    for bk, cap in enumerate(caps):
        boff.append(scol)
        left = cap
        while left > 0:
            n0 = min(MAX_CALL, left)
            n1 = min(MAX_CALL, left - n0)
            pairs.append((bk, n0, n1, icol, scol))
            icol += (n0 + n1) // 16
            scol += (n0 + n1) // 128
            left -= n0 + n1
    return pairs, icol, scol, boff


# ---------------------------------------------------------------- device code
def build_bass(caps, ep0):
    """caps: per-bucket capacities (edge slots, multiples of 128), shared by
    all cores (max over cores).  ep0: bucket-0 pairs gathering only the
    HROWS-row prefix of p1 block 0 (host orders those edges first)."""
    calls, idx_cols, tot_slots, boff = _plan_calls(caps)
    ncall = len(calls)

    nc = bass.Bass(dynamic_dma_scratch_size=32768, num_swdge_queues=2)

    e1t = nc.dram_tensor("e1t", [D_IN, P1_ROWS], BF16, kind="ExternalInput")
    e2t = nc.dram_tensor("e2t", [D_IN, P2_ROWS], BF16, kind="ExternalInput")
    w12 = nc.dram_tensor("w12", [D_IN, 2 * D_OUT], BF16, kind="ExternalInput")
    bo3 = nc.dram_tensor("bo3", [3, D_OUT], BF16, kind="ExternalInput")
    idxa = nc.dram_tensor("idxa", [128, idx_cols], I16, kind="ExternalInput")
    idxb = nc.dram_tensor("idxb", [128, idx_cols], I16, kind="ExternalInput")
    res = nc.dram_tensor("res", [128, tot_slots], F32, kind="ExternalOutput")

    p1d = nc.dram_tensor("p1d", [P1_ROWS, D_OUT], BF16, kind="Internal")
    p2d = nc.dram_tensor("p2d", [P2_ROWS, D_OUT], BF16, kind="Internal")
    pdst = (p1d, p2d)

    st = contextlib.ExitStack()
    with st:
        sb = lambda nm, shape, dt: st.enter_context(nc.sbuf_tensor(nm, shape, dt))
        sem = lambda nm: st.enter_context(nc.semaphore(name=nm))

        # wc columns: [w1_k0 | w2_k0 | w1_k1 | w2_k1] (128 each)
        wc = sb("wc", [128, 512], BF16)
        bt = [sb(f"bt{i}", [1, 128], BF16) for i in range(3)]  # b1, b2, ones
        idxt = (sb("idxta", [128, idx_cols], I16), sb("idxtb", [128, idx_cols], I16))
        et = [[sb(f"et_{p}_{k}", [128, EMB_COLS], BF16) for k in range(2)]
              for p in range(3)]  # [buffer cq%3][k-half]
        pvt = sb("pvt", [128, 4 * 512], BF16)
        pv = [pvt[:, i * 512:(i + 1) * 512] for i in range(4)]
        ps = [st.enter_context(nc.psum_tensor(f"ps{i}", [128, 512], F32))
              for i in range(4)]
        # Gather pairs are merged into "items" of up to two full pairs (4096
        # edges) sharing one contiguous super-slot, so each DVE chain op's
        # fixed cost is amortized over twice the edges.  4 rotating slots.
        NBUF = 5
        ITEM_COLS = 4 * MAX_CALL
        atb = sb("atb", [128, NBUF * ITEM_COLS], BF16)
        btgb = sb("btgb", [128, NBUF * ITEM_COLS], BF16)
        rt = sb("rt", [128, tot_slots], F32)

        s_cl = sem("s_cl")               # w const loads (2 dmas -> 32)
        s_clb = sem("s_clb")             # bias/ones const loads (3 -> 48)
        s_cli = sem("s_cli")             # idx table loads (2 dmas -> 32)
        s_load = tuple(sem(f"s_load{i}") for i in range(3))  # SP embT, by cq%3
        s_loadp = sem("s_loadp")         # Pool embT loads
        s_mm = sem("s_mm")               # matmuls (+1 each; 3 per tile)
        s_cp = sem("s_cp")               # act copies (+1 each)
        s_cpd = sem("s_cpd")             # dve copies (+1 each)
        s_pw = tuple(sem(f"s_pw{i}") for i in range(2))  # pair writes, by j%2
        s_pwx = sem("s_pwx")             # first halves of crossing pairs
        # gather-completion sems by item%NBUF per queue: item w+NBUF's gathers
        # are gated on s_red >= w+1, so updates of each sem are causally
        # ordered.
        s_ga = tuple(sem(f"s_ga{i}") for i in range(NBUF))  # queue-0, +16/call
        s_gb = tuple(sem(f"s_gb{i}") for i in range(NBUF))  # queue-1, +16/call
        s_st = tuple(sem(f"s_st{i}") for i in range(4))  # DVE dot stages 0-3
        s_red = sem("s_red")             # final reduces (+1 per item)
        s_out = sem("s_out")             # res dma

        CONSTS = 2 * 16

        # items: (pair list, slot columns per pair member, total slots, scol)
        items = []
        _p = 0
        while _p < ncall:
            bk0, n0, n1, icol0, scol0 = calls[_p]
            if (n0 + n1 == 2 * MAX_CALL and _p + 1 < ncall
                    and calls[_p + 1][0] == bk0):
                items.append(([_p, _p + 1], scol0))
                _p += 2
            else:
                items.append(([_p], scol0))
                _p += 1
        nitem = len(items)
        item_slots = [sum((calls[p][1] + calls[p][2]) // 128 for p in ps_)
                      for ps_, _ in items]

        # per-sem cumulative gather counts per item
        _sem_cnt = [0] * NBUF
        gwait = []
        for _w, (_ps, _sc) in enumerate(items):
            _sem_cnt[_w % NBUF] += 16 * sum(
                2 if calls[p][2] else 1 for p in _ps)
            gwait.append(_sem_cnt[_w % NBUF])

        # During the pre-gather phase the DVE is mostly idle: it takes the odd
        # groups' psum->pv copies for the first PH1 groups.
        dve_copy = [q < PH1 and q % 2 == 1 for q in range(NGROUP)]
        cp_cnt = []  # cumulative (act, dve) copy counts through group q
        _na = _ndv = 0
        for q in range(NGROUP):
            if dve_copy[q]:
                _ndv += 1
            else:
                _na += 1
            cp_cnt.append((_na, _ndv))

        def _copy_wait(eng, q):
            """wait until the copy of group q is complete."""
            if dve_copy[q]:
                eng.wait_ge(s_cpd, cp_cnt[q][1])
            else:
                eng.wait_ge(s_cp, cp_cnt[q][0])

        block = st.enter_context(nc.Block())

        # Chunks 1,3,5,7,9 are loaded by the Pool engine during its pre-gather
        # idle window (own sem s_loadp); SP loads the rest (s_load by parity).
        POOL_CHUNKS = (4, 6, 8)
        chunk_wait = {}
        _cnt = {0: 0, 1: 0, 2: 0, 'p': 0}
        for _cq in range(NCHUNK):
            key = 'p' if _cq in POOL_CHUNKS else _cq % 3
            _cnt[key] += 32
            chunk_wait[_cq] = (key, _cnt[key])

        # p-writes happen per PAIR of groups (2j, 2j+1): pv slots are adjacent
        # so one DMA covers both, halving Act's per-write descriptor-gen
        # floor.  Pairs whose two groups land in different projection blocks
        # ("crossing": non-contiguous table rows) are written as two singles,
        # the first tracked by s_pwx.
        NPAIR_W = NGROUP // 2
        CROSSING = frozenset(j for j in range(NPAIR_W)
                             if GSEQ[2 * j + 1] != GSEQ[2 * j] + 1)
        _xrank = {j: i + 1 for i, j in enumerate(sorted(CROSSING))}

        def _one_write(eng, q, s_sem, cnt):
            g = GSEQ[q]
            tab = 0 if g < GROUPS1 else 1
            r0 = g * 512 if tab == 0 else (g - GROUPS1) * 512
            # (p t)-permuted write: table row r0 + p*4 + t <- node r0 + t*128 + p
            eng.dma_start(
                out=pdst[tab][r0:r0 + 512, :].rearrange("(p t) d -> p t d", p=128),
                in_=pv[q % 4][:].rearrange("p (t d) -> p t d", d=128),
            ).then_inc(s_sem, cnt)

        def p_write_pair(eng, j):
            if j in CROSSING:
                _one_write(eng, 2 * j, s_pwx, 16)
                _one_write(eng, 2 * j + 1, s_pw[j % 2], 16)
                return
            q0 = 2 * j
            g = GSEQ[q0]
            tab = 0 if g < GROUPS1 else 1
            r0 = g * 512 if tab == 0 else (g - GROUPS1) * 512
            s0 = q0 % 4
            eng.dma_start(
                out=pdst[tab][r0:r0 + 1024, :]
                    .rearrange("(g p t) d -> p g t d", p=128, t=4),
                in_=pvt[:, s0 * 512:(s0 + 2) * 512]
                    .rearrange("p (g t d) -> p g t d", t=4, d=128),
            ).then_inc(s_pw[j % 2], 16)

        def wait_pairs_through(eng, nq):
            """wait until all p-writes for groups < nq are complete (nq is
            rounded up to a pair boundary)."""
            npair = (nq + 1) // 2
            for r in range(2):
                eng.wait_ge(s_pw[r], 16 * len(range(r, npair, 2)))
            nx = sum(1 for j in CROSSING if j < npair)
            if nx:
                eng.wait_ge(s_pwx, 16 * nx)

        def wait_pv_drained(eng, q):
            """wait until pv slot q%4 (last used by group q-4) is rewritable."""
            p4 = (q - 4) // 2
            eng.wait_ge(s_pw[p4 % 2], 16 * (p4 // 2 + 1))
            if p4 in CROSSING:
                eng.wait_ge(s_pwx, 16 * _xrank[p4])

        def load_chunk(eng, cq):
            if cq >= 3:
                # buffer cq%3 previously held chunk cq-3; wait consumed
                eng.wait_ge(s_mm, 3 * CHUNK_T * (cq - 2))
            tab, col0 = _chunk_src(CSEQ[cq])
            src = e1t if tab == 0 else e2t
            par = cq % 3
            s_ld = s_loadp if cq in POOL_CHUNKS else s_load[cq % 3]
            eng.dma_start(out=et[par][0][:],
                          in_=src[0:128, col0:col0 + EMB_COLS]).then_inc(s_ld, 16)
            eng.dma_start(out=et[par][1][:],
                          in_=src[128:256, col0:col0 + EMB_COLS]).then_inc(s_ld, 16)

        # ------------------------------------------------ SP: embT + p-writes
        @block.sync
        def _(sync):
            for k in range(2):
                sync.dma_start(out=wc[:, k * 256:(k + 1) * 256],
                               in_=w12[k * 128:(k + 1) * 128, :]).then_inc(s_cl, 16)
            load_chunk(sync, 0)
            for i in range(3):
                sync.dma_start(out=bt[i][:], in_=bo3[i:i + 1, :]).then_inc(s_clb, 16)
            load_chunk(sync, 1)
            next_cq = 2
            for q in range(NGROUP):
                # look ahead: issue loads for chunks starting within 5 groups
                while next_cq < NCHUNK and next_cq * CHUNK_T <= (q + 5) * 4 + 3:
                    if next_cq not in POOL_CHUNKS:
                        load_chunk(sync, next_cq)
                    next_cq += 1


        # ------------------------------------------------ PE: projections
        @block.tensor
        def _(tensor):
            tensor.wait_ge(s_cl, CONSTS)
            for q, g in enumerate(GSEQ):
                tab = 0 if g < GROUPS1 else 1
                bc = bt[tab][:]
                for j in range(4):
                    tq = q * 4 + j
                    cid, col0 = _chunk_of_tile(g * 4 + j)
                    cq = CPOS[cid]
                    if tq == cq * CHUNK_T:  # first processed tile of chunk
                        key, cntv = chunk_wait[cq]
                        tensor.wait_ge(s_loadp if key == 'p' else s_load[key], cntv)
                    if j == 0 and q >= 4:
                        _copy_wait(tensor, q - 4)  # psum bank q%4 free
                    out = ps[q % 4][:, j * 128:(j + 1) * 128]
                    tensor.matmul(out=out, lhsT=et[cq % 3][0][:, col0:col0 + 128],
                                  rhs=wc[:, tab * 128:tab * 128 + 128],
                                  start=True, stop=False).then_inc(s_mm, 1)
                    tensor.matmul(out=out, lhsT=et[cq % 3][1][:, col0:col0 + 128],
                                  rhs=wc[:, 256 + tab * 128:256 + tab * 128 + 128],
                                  start=False, stop=False).then_inc(s_mm, 1)
                    if tq == 0:
                        tensor.wait_ge(s_clb, 48)
                    tensor.matmul(out=out, lhsT=bt[2][:], rhs=bc,
                                  start=False, stop=True).then_inc(s_mm, 1)

        # ------------------------------------------------ Act: psum->sbuf + DMAs
        @block.scalar
        def _(scalar):
            scalar.wait_ge(s_cl, CONSTS)
            # the pair write of groups (q-2, q-1) is issued after the copy of
            # group q so the copy-completion sems are already visible.
            for q, g in enumerate(GSEQ):
                if not dve_copy[q]:
                    scalar.wait_ge(s_mm, 12 * q + 12)
                    if q >= 4:
                        wait_pv_drained(scalar, q)
                    scalar.copy(out=pv[q % 4][:], in_=ps[q % 4][:]).then_inc(s_cp, 1)
                if q >= 2 and q % 2 == 0:
                    _copy_wait(scalar, q - 2)
                    _copy_wait(scalar, q - 1)
                    p_write_pair(scalar, (q - 2) // 2)
            _copy_wait(scalar, NGROUP - 2)
            _copy_wait(scalar, NGROUP - 1)
            p_write_pair(scalar, NPAIR_W - 1)
            # write results in thirds so only the last sliver is exposed
            cut1, cut2 = nitem // 3, (2 * nitem) // 3
            s1, s2 = items[cut1][1], items[cut2][1]
            scalar.wait_ge(s_red, cut1)
            scalar.dma_start(out=res[:, :s1], in_=rt[:, :s1]).then_inc(s_out, 16)
            scalar.wait_ge(s_red, cut2)
            scalar.dma_start(out=res[:, s1:s2], in_=rt[:, s1:s2]).then_inc(s_out, 16)
            scalar.wait_ge(s_red, nitem)
            scalar.dma_start(out=res[:, s2:], in_=rt[:, s2:]).then_inc(s_out, 16)
            scalar.wait_ge(s_out, 48)

        # ------------------------------------------------ Pool: gathers
        @block.gpsimd
        def _(gpsimd):
            # the Pool engine is idle until the first bucket gate (~40us): it
            # loads the consts, every other early embT chunk, and its own idx
            # tables, so SP's chunk stream (which feeds the PE) never blocks.
            gpsimd.dma_start(out=idxt[0][:], in_=idxa[:]).then_inc(s_cli, 16)
            gpsimd.dma_start(out=idxt[1][:], in_=idxb[:]).then_inc(s_cli, 16)
            for i, cq in enumerate(POOL_CHUNKS):
                if i:
                    gpsimd.wait_ge(s_loadp, 32 * i)  # order s_loadp updates
                load_chunk(gpsimd, cq)
            from concourse import library_config
            gpsimd.load_library(library_config.mlp)
            sizes = ({c[1] for c in calls} | {c[2] for c in calls}) - {0}
            regs = {n: gpsimd.to_reg(n) for n in sorted(sizes)}
            gpsimd.wait_ge(s_cl, CONSTS)
            gpsimd.wait_ge(s_cli, 32)

            # first ep0 pairs of bucket 0 reference only p1 rows < HROWS (the
            # host orders bucket-0 edges early-rows-first), so they unblock
            # after 25 + HROWS/512 projected groups instead of 50.
            gate_req = []
            for p, (bk, n0, n1, icol, scol) in enumerate(calls):
                if bk == 0 and p < ep0:
                    gate_req.append(GPB + HROWS // 512)
                else:
                    gate_req.append(BK_READY_Q[bk])

            cur_gate = -1
            for w, (ps_, scol_w) in enumerate(items):
                if w >= NBUF:
                    gpsimd.wait_ge(s_red, w - NBUF + 1)  # slot w%NBUF consumed
                base = (w % NBUF) * ITEM_COLS
                nh = 0
                for p in ps_:
                    bk, n0, n1, icol, scol = calls[p]
                    fi, ti = bk // NTB, bk % NTB
                    if gate_req[p] > cur_gate:
                        cur_gate = gate_req[p]
                        wait_pairs_through(gpsimd, cur_gate)
                    rows1 = HROWS if (bk == 0 and p < ep0) else NBP
                    # rows move as 64 packed f32 (= 128 bf16): same bytes and
                    # descriptors, half the modeled element count.
                    for n_i, n in enumerate((n0, n1)):
                        if n == 0:
                            continue
                        S = n // 128
                        c0 = icol + (nh % (2 * MAX_CALL)) // 16
                        gpsimd.dma_gather(
                            out_ap=atb[:, base + nh:base + nh + S * 128]
                                .bitcast(F32).rearrange("p (s d) -> p s d", d=64),
                            in_ap=p1d[fi * NBP:fi * NBP + rows1, :].bitcast(F32),
                            idxs_ap=idxt[0][:, c0:c0 + n // 16],
                            num_idxs=n, num_idxs_reg=regs[n],
                            elem_size=D_OUT // 2, queue_num=0,
                        ).then_inc(s_ga[w % NBUF], 16)
                        gpsimd.dma_gather(
                            out_ap=btgb[:, base + nh:base + nh + S * 128]
                                .bitcast(F32).rearrange("p (s d) -> p s d", d=64),
                            in_ap=p2d[ti * NBP:(ti + 1) * NBP, :].bitcast(F32),
                            idxs_ap=idxt[1][:, c0:c0 + n // 16],
                            num_idxs=n, num_idxs_reg=regs[n],
                            elem_size=D_OUT // 2, queue_num=1,
                        ).then_inc(s_gb[w % NBUF], 16)
                        nh += S * 128

        # ------------------------------------------------ DVE: dot products
        # bf16 multiply (2x mode), tree-halve in bf16 (2x) down to 16 partials
        # per slot, then one short f32 reduce - one chain per gather pair
        # (up to 2048 edges).  The 5-stage chain is software-pipelined across
        # pairs (stage s of pair p in round p+s) so every intra-chain
        # semaphore is already visible when its wait issues.
        @block.vector
        def _(vector):
            # phase-1 assist: odd-group psum->pv copies while gathers are gated
            for q in range(NGROUP):
                if not dve_copy[q]:
                    continue
                vector.wait_ge(s_mm, 12 * q + 12)
                if q >= 4:
                    wait_pv_drained(vector, q)
                vector.tensor_copy(out=pv[q % 4][:], in_=ps[q % 4][:]).then_inc(s_cpd, 1)

            def stage(s, w):
                S = item_slots[w]
                scol_w = items[w][1]
                base = (w % NBUF) * ITEM_COLS
                a3 = atb[:, base:base + S * 128]
                av = a3.rearrange("p (s d) -> p s d", d=128)
                if s == 0:
                    vector.wait_ge(s_ga[w % NBUF], gwait[w])
                    vector.wait_ge(s_gb[w % NBUF], gwait[w])
                    vector.tensor_mul(out=a3, in0=a3,
                                      in1=btgb[:, base:base + S * 128]
                                      ).then_inc(s_st[0], 1)
                elif s in (1, 2, 3):
                    h = 128 >> s  # 64, 32, 16
                    vector.wait_ge(s_st[s - 1], w + 1)
                    vector.tensor_add(out=av[:, :, 0:h], in0=av[:, :, 0:h],
                                      in1=av[:, :, h:2 * h]).then_inc(s_st[s], 1)
                else:
                    vector.wait_ge(s_st[3], w + 1)
                    vector.reduce_sum(out=rt[:, scol_w:scol_w + S],
                                      in_=av[:, :, 0:16], axis=AX.X).then_inc(s_red, 1)

            for r in range(nitem + 4):
                for s in range(4, -1, -1):
                    w = r - s
                    if 0 <= w < nitem:
                        stage(s, w)

    return nc, calls, boff


_NC_CACHE: dict = {}


def _get_nc(caps):
    caps, ep0 = caps
    key = (tuple(caps), ep0)
    if key not in _NC_CACHE:
        nc, calls, boff = build_bass(caps, ep0)
        from concourse.library_overlay import lower_extended_insts
        lower_extended_insts(nc)
        _NC_CACHE[key] = (nc, calls, boff)
    return _NC_CACHE[key]


# ---------------------------------------------------------------- host side
def _perm_local(n):
    """block-local node id -> permuted table row (within block).
    Table row g*512 + p*4 + t holds node g*512 + t*128 + p."""
    g, u = np.divmod(n, 512)
    t, p = np.divmod(u, 128)
    return g * 512 + p * 4 + t


def _marshal(emb_1, emb_2, nodes_from_to, W1, b1, W2, b2):
    """Shard/bucket inputs per core.  Returns (caps, in_maps, books)."""
    import ml_dtypes
    bf16 = ml_dtypes.bfloat16

    f = np.asarray(nodes_from_to[:, 0], dtype=np.int64)
    t = np.asarray(nodes_from_to[:, 1], dtype=np.int64)
    emb_1 = np.asarray(emb_1, dtype=np.float32)
    emb_2 = np.asarray(emb_2, dtype=np.float32)
    w12 = np.concatenate(
        [np.asarray(W1, dtype=np.float32), np.asarray(W2, dtype=np.float32)],
        axis=1).astype(bf16)
    bo3 = np.stack([
        np.asarray(b1, dtype=np.float32).reshape(-1),
        np.asarray(b2, dtype=np.float32).reshape(-1),
        np.ones(D_OUT, np.float32),
    ]).astype(bf16)

    core = (f // (NFB * NB)) * 4 + t // (NTB * NB)
    order0 = np.argsort(core, kind="stable")
    ccnt = np.bincount(core, minlength=N_CORES)
    coff = np.concatenate([[0], np.cumsum(ccnt)])

    percore = []
    early_cnts = []
    all_cnts = np.zeros((N_CORES, NBUCKET), np.int64)
    for c in range(N_CORES):
        a, b = c // 4, c % 4
        sel = order0[coff[c]:coff[c + 1]]
        fc, tcv = f[sel], t[sel]
        fi = fc // NB - NFB * a
        ti = tcv // NB - NTB * b
        fl = _perm_local(fc % NB).astype(np.int16)
        tl = _perm_local(tcv % NB).astype(np.int16)
        bk = fi * NTB + ti
        o2 = np.argsort(bk, kind="stable")
        sel2, fl2, tl2 = sel[o2], fl[o2], tl[o2]
        cnts = np.bincount(bk, minlength=NBUCKET)
        all_cnts[c] = cnts
        # bucket 0: early (prefix-row) edges first, enabling the half-gate
        n0 = cnts[0]
        e0 = fl2[:n0] < HROWS
        o3 = np.argsort(~e0, kind="stable")
        sel2[:n0], fl2[:n0], tl2[:n0] = sel2[:n0][o3], fl2[:n0][o3], tl2[:n0][o3]
        early_cnts.append(int(e0.sum()))
        percore.append((a, b, sel2, fl2, tl2, cnts))

    caps = [int(-(-all_cnts[:, k].max() // 128) * 128) for k in range(NBUCKET)]
    ep0 = min(early_cnts) // (2 * MAX_CALL)
    calls, idx_cols, tot_slots, boff = _plan_calls(caps)

    in_maps, books = [], []
    for c in range(N_CORES):
        a, b, sel2, fl2, tl2, cnts = percore[c]
        pos = np.concatenate([[0], np.cumsum(cnts)])

        slots_a = np.zeros((NBUCKET, max(caps)), np.int16)
        slots_b = np.zeros((NBUCKET, max(caps)), np.int16)
        for k in range(NBUCKET):
            slots_a[k, :cnts[k]] = fl2[pos[k]:pos[k + 1]]
            slots_b[k, :cnts[k]] = tl2[pos[k]:pos[k + 1]]
        # wrap by 16: idx i of a bucket at (partition i%16, col i//16),
        # replicated across the 8 groups of 16 partitions
        wa_cols = []
        wb_cols = []
        for k in range(NBUCKET):
            cap = caps[k]
            wa_cols.append(slots_a[k, :cap].reshape(cap // 16, 16).T)
            wb_cols.append(slots_b[k, :cap].reshape(cap // 16, 16).T)
        idxa = np.tile(np.concatenate(wa_cols, axis=1), (8, 1))
        idxb = np.tile(np.concatenate(wb_cols, axis=1), (8, 1))

        e1t = np.zeros((D_IN, P1_ROWS), bf16)
        for i in range(NFB):
            blk = emb_1[(NFB * a + i) * NB:(NFB * a + i + 1) * NB]
            e1t[:, i * NBP:i * NBP + NB] = blk.T.astype(bf16)
        e2t = np.zeros((D_IN, P2_ROWS), bf16)
        for i in range(NTB):
            blk = emb_2[(NTB * b + i) * NB:(NTB * b + i + 1) * NB]
            e2t[:, i * NBP:i * NBP + NB] = blk.T.astype(bf16)

        in_maps.append({
            "e1t": e1t, "e2t": e2t, "w12": w12, "bo3": bo3,
            "idxa": np.ascontiguousarray(idxa),
            "idxb": np.ascontiguousarray(idxb),
        })
        books.append((sel2, cnts, pos))
    return (caps, ep0), in_maps, books


def _unmarshal(results, books, caps, n_edges):
    calls, idx_cols, tot_slots, boff = _plan_calls(caps[0])
    out = np.empty(n_edges, np.float32)
    for c in range(N_CORES):
        sel2, cnts, pos = books[c]
        r = results[c]["res"]  # [128, tot_slots]
        for k in range(NBUCKET):
            if cnts[k] == 0:
                continue
            s0 = boff[k]
            nslots = caps[0][k] // 128
            stream = r[:, s0:s0 + nslots].T.reshape(-1)
            out[sel2[pos[k]:pos[k + 1]]] = stream[:cnts[k]]
    return out


def _run(inputs, trace=False, **run_kwargs):
    from concourse.bass_utils import run_bass_kernel_spmd

    caps, in_maps, books = _marshal(**inputs)
    nc, calls, boff = _get_nc(caps)
    r = run_bass_kernel_spmd(
        nc, in_maps, core_ids=list(range(N_CORES)), trace=trace, **run_kwargs
    )
    out = _unmarshal(r.results, books, caps, len(inputs["nodes_from_to"]))
    return out, r


def kernel(**inputs) -> np.ndarray:
    out, _ = _run(inputs, trace=False)
    return out
